# revision 9
# baseline (speedup 1.0000x reference)
"""Trainium2 Bass kernel for BRT fused experts (grouped GEMM pair, no activation).

Reference semantics (per expert e):
    h   = x[e] @ wi_w[e].T + wi_b[e]        # [C, H] @ [H, I] -> [C, I]
    out = h @ wo_w[e].T + wo_b[e]           # [C, I] @ [I, H] -> [C, H]

Full dims: E=16, B=1, C=64, H=2048, I=8192, fp16.

Strategy: expert-parallel over 8 cores (2 experts/core), SPMD. Host
pre-transposes weights so the contraction dim is on SBUF partitions;
device streams weights (134 MB/core) at full DMA rate — the kernel is
HBM-bandwidth-bound (~320-375 us/core roofline at 360-420 GB/s).

Per expert on-device:
  fc1: lhsT = xT chunks [128, 64] (stationary), rhs = wiT tiles
       [128, 2048] (4 KB contiguous rows — measured 420 GB/s vs ~380
       for 2 KB rows), accumulate [64, 1024] in two PSUM banks over 16
       K-chunks; bias added via a one-hot-selector matmul.
  transpose: PE-transpose h [64, I] -> hT tiles [128, 64] (identity matmul).
  fc2: lhsT = hT chunks [128, 64], rhs = woT tiles [128, 2048],
       accumulate [64, 2048] in 4 PSUM banks over 64 K-chunks + bias.

Bias trick: SEL [128, G*64] holds one-hot column blocks (block g has row
g = 1, rest 0). matmul(ps, SEL[:, g*64:(g+1)*64], vals[:, :512]) adds
vals[g, :] (= bias[g*512:(g+1)*512]) to every output row. Per-expert
vals tiles kill cross-expert WAR chains on the in-order SP DMA queue.

Output DMA goes out on the ACT HWDGE ring (nc.scalar.dma_start) so its
dependency on the PSUM drain never stalls the SP weight-stream ring.
"""

import os
from contextlib import ExitStack

import numpy as np

E, B, C, H, I = 16, 1, 64, 2048, 8192
N_CORES = 8
E_LOC = E // N_CORES

_CACHE = {}


def build_program(e_loc=E_LOC, c=C, h=H, i=I, wi_bufs=25, wo_bufs=10):
    import concourse.bass as bass
    import concourse.tile as tile
    from concourse import bacc, mybir
    from concourse.masks import make_identity

    fp16 = mybir.dt.float16
    fp32 = mybir.dt.float32

    assert c == 64 and h % 512 == 0 and i % 1024 == 0
    KH = h // 128          # fc1 contraction chunks
    KI = i // 128          # fc2 contraction chunks
    WI_TILE = 2048 if i % 2048 == 0 else 1024
    N_IGRP = i // WI_TILE
    PS1_W = 1024           # fc1 psum block width (2 banks)
    SUBS = WI_TILE // PS1_W
    NH = h // 512          # fc2 output column blocks
    TP_PER = 16 if KI % 16 == 0 else KI   # transposes per psum staging tile
    N_TGRP = KI // TP_PER
    G_WI = i // 512        # bias selector blocks for fc1
    G = max(G_WI, NH)

    nc = bacc.Bacc(
        "TRN2",
        target_bir_lowering=False,
        debug=False,
        enable_asserts=False,
        num_devices=N_CORES,
    )

    xt_ap = nc.dram_tensor("xt", [e_loc, 128, KH * c], fp16, kind="ExternalInput").ap()
    wiT_ap = nc.dram_tensor("wiT", [e_loc, h, i], fp16, kind="ExternalInput").ap()
    wib_ap = nc.dram_tensor("wib", [e_loc, G_WI, 512], fp16, kind="ExternalInput").ap()
    woT_ap = nc.dram_tensor("woT", [e_loc, i, h], fp16, kind="ExternalInput").ap()
    wob_ap = nc.dram_tensor("wob", [e_loc, NH, 512], fp16, kind="ExternalInput").ap()
    out_ap = nc.dram_tensor("out", [e_loc, c, h], fp16, kind="ExternalOutput").ap()

    with tile.TileContext(nc) as tc, ExitStack() as ctx:
        const_pool = ctx.enter_context(tc.tile_pool(name="const", bufs=1))
        xt_pool = ctx.enter_context(tc.tile_pool(name="xt", bufs=2))
        wi_pool = ctx.enter_context(tc.tile_pool(name="wi", bufs=wi_bufs))
        wo_pool = ctx.enter_context(tc.tile_pool(name="wo", bufs=wo_bufs))
        h_pool = ctx.enter_context(tc.tile_pool(name="h", bufs=1))
        ht_pool = ctx.enter_context(tc.tile_pool(name="ht", bufs=1))
        osb_pool = ctx.enter_context(tc.tile_pool(name="osb", bufs=2))
        ps1_pool = ctx.enter_context(tc.tile_pool(name="ps1", bufs=2, space="PSUM"))
        ps2_pool = ctx.enter_context(tc.tile_pool(name="ps2", bufs=1, space="PSUM"))

        ident = const_pool.tile([128, 128], fp16, tag="ident")
        make_identity(nc, ident)

        # one-hot selector: column block g has row g = 1, all else 0
        sel = const_pool.tile([128, G * c], fp16, tag="sel")
        nc.gpsimd.memset(sel, 0.0)
        sel3 = sel.rearrange("p (g c) -> p g c", c=c)
        nc.gpsimd.affine_select(
            out=sel3,
            in_=sel3,
            compare_op=mybir.AluOpType.not_equal,
            fill=1.0,
            base=0,
            # iota(p, g, c) = p - g; rows where p == g get fill=1.0
            pattern=[[-1, G], [0, c]],
            channel_multiplier=1,
        )

        # per-expert packed bias values (row g = bias[g*512:(g+1)*512])
        bwi = []
        bwo = []
        for e in range(e_loc):
            t = const_pool.tile([128, 512], fp16, tag=f"bwi{e}")
            nc.gpsimd.memset(t[:], 0.0)
            nc.gpsimd.dma_start(t[0:G_WI, :], wib_ap[e])
            bwi.append(t)
            t = const_pool.tile([128, 512], fp16, tag=f"bwo{e}")
            nc.gpsimd.memset(t[:], 0.0)
            nc.gpsimd.dma_start(t[0:NH, :], wob_ap[e])
            bwo.append(t)

        # both experts' activations up front (SWDGE; keeps the SP HWDGE
        # ring 100% weight traffic so its flow-control lanes never stall)
        xts = []
        for e in range(e_loc):
            xt_sb = xt_pool.tile([128, KH * c], fp16, tag="xt")
            nc.gpsimd.dma_start(xt_sb[:], xt_ap[e])
            xts.append(xt_sb)

        for e in range(e_loc):
            xt_sb = xts[e]
            h_sb = h_pool.tile([c, i], fp16, tag="h")

            # ---- fc1: h = x @ wiT + bi ----
            # Two live [64, PS1_W] accumulators per weight group so every
            # wi tile is fully consumed (all SUBS column blocks) the moment
            # it arrives — slot frees track DMA pace instead of bursting at
            # group end (which starved the DMA ~8 us/group).
            for ig in range(N_IGRP):
                pss = [ps1_pool.tile([c, PS1_W], fp32, tag="ps1",
                                     name=f"ps1_{e}_{ig}_{s}")
                       for s in range(SUBS)]
                for k in range(KH):
                    wt = wi_pool.tile([128, WI_TILE], fp16, tag="wi")
                    nc.sync.dma_start(
                        wt[:],
                        wiT_ap[e, k * 128 : (k + 1) * 128,
                               ig * WI_TILE : (ig + 1) * WI_TILE],
                    )
                    for sub in range(SUBS):
                        for q in range(PS1_W // 512):
                            nc.tensor.matmul(
                                pss[sub][:, q * 512 : (q + 1) * 512],
                                xt_sb[:, k * c : (k + 1) * c],
                                wt[:, sub * PS1_W + q * 512
                                   : sub * PS1_W + (q + 1) * 512],
                                start=(k == 0),
                                stop=False,
                            )
                for sub in range(SUBS):
                    off = ig * WI_TILE + sub * PS1_W
                    for q in range(PS1_W // 512):
                        g = off // 512 + q
                        nc.tensor.matmul(
                            pss[sub][:, q * 512 : (q + 1) * 512],
                            sel[:, g * c : (g + 1) * c],
                            bwi[e][:],
                            start=False,
                            stop=True,
                        )
                    nc.scalar.copy(h_sb[:, off : off + PS1_W], pss[sub][:])

            # ---- transpose h -> hT ----
            ht_sb = ht_pool.tile([128, KI * c], fp16, tag="ht")
            for tg in range(N_TGRP):
                pst = ps1_pool.tile([128, TP_PER * c], fp16, tag="ps1")
                for j in range(TP_PER):
                    jj = tg * TP_PER + j
                    nc.tensor.transpose(
                        pst[:, j * c : (j + 1) * c],
                        h_sb[:, jj * 128 : (jj + 1) * 128],
                        ident[:c, :c],
                    )
                nc.vector.tensor_copy(
                    ht_sb[:, tg * TP_PER * c : (tg + 1) * TP_PER * c], pst[:]
                )

            # ---- fc2: out = h @ woT + bo ----
            pso = ps2_pool.tile([c, h], fp32, tag="ps2")
            for k in range(KI):
                wot = wo_pool.tile([128, h], fp16, tag="wo")
                nc.sync.dma_start(wot[:], woT_ap[e, k * 128 : (k + 1) * 128, :])
                for n in range(NH):
                    nc.tensor.matmul(
                        pso[:, n * 512 : (n + 1) * 512],
                        ht_sb[:, k * c : (k + 1) * c],
                        wot[:, n * 512 : (n + 1) * 512],
                        start=(k == 0),
                        stop=False,
                    )
            for n in range(NH):
                nc.tensor.matmul(
                    pso[:, n * 512 : (n + 1) * 512],
                    sel[:, n * c : (n + 1) * c],
                    bwo[e][:],
                    start=False,
                    stop=True,
                )
                out_sb = osb_pool.tile([c, 512], fp16, tag="osb")
                nc.vector.tensor_copy(out_sb[:], pso[:, n * 512 : (n + 1) * 512])
                nc.gpsimd.dma_start(out_ap[e, :, n * 512 : (n + 1) * 512], out_sb[:])

    nc.compile()
    return nc


def _get_program():
    key = (E_LOC, C, H, I)
    if key not in _CACHE:
        _CACHE[key] = build_program()
    return _CACHE[key]


def _make_in_maps(inputs, wi_w, wi_b, wo_w, wo_b):
    x = np.asarray(inputs, dtype=np.float16).reshape(E, C, H)
    # xt[e, p, k*C+c] = x[e, c, k*128+p]
    xt = np.ascontiguousarray(
        x.transpose(0, 2, 1).reshape(E, H // 128, 128, C)
        .transpose(0, 2, 1, 3).reshape(E, 128, (H // 128) * C)
    )
    wiT = np.ascontiguousarray(
        np.asarray(wi_w, dtype=np.float16).transpose(0, 2, 1)
    )  # [E, H, I]
    woT = np.ascontiguousarray(
        np.asarray(wo_w, dtype=np.float16).transpose(0, 2, 1)
    )  # [E, I, H]
    wib = np.ascontiguousarray(np.asarray(wi_b, dtype=np.float16)).reshape(E, I // 512, 512)
    wob = np.ascontiguousarray(np.asarray(wo_b, dtype=np.float16)).reshape(E, H // 512, 512)

    in_maps = []
    for r in range(N_CORES):
        s = slice(r * E_LOC, (r + 1) * E_LOC)
        in_maps.append(
            {
                "xt": np.ascontiguousarray(xt[s]),
                "wiT": np.ascontiguousarray(wiT[s]),
                "wib": np.ascontiguousarray(wib[s]),
                "woT": np.ascontiguousarray(woT[s]),
                "wob": np.ascontiguousarray(wob[s]),
            }
        )
    return in_maps


def run(inputs, wi_w, wi_b, wo_w, wo_b, trace=False):
    """Returns (output [E,B,C,H] fp16, exec_time_ns or None)."""
    from concourse.bass_utils import run_bass_kernel_spmd

    nc = _get_program()
    in_maps = _make_in_maps(inputs, wi_w, wi_b, wo_w, wo_b)
    res = run_bass_kernel_spmd(nc, in_maps, list(range(N_CORES)), trace=trace)
    out = np.stack([res.results[r]["out"] for r in range(N_CORES)])
    out = out.reshape(E, B, C, H).astype(np.float16)
    return out, res.exec_time_ns


def kernel(inputs, wi_w, wi_b, wo_w, wo_b):
    out, _ = run(inputs, wi_w, wi_b, wo_w, wo_b, trace=False)
    return out


# revision 13
# speedup vs baseline: 1.1413x; 1.1413x over previous
"""Trainium2 Bass kernel for BRT fused experts (grouped GEMM pair, no activation).

Reference semantics (per expert e):
    h   = x[e] @ wi_w[e].T + wi_b[e]        # [C, H] @ [H, I] -> [C, I]
    out = h @ wo_w[e].T + wo_b[e]           # [C, I] @ [I, H] -> [C, H]

Full dims: E=16, B=1, C=64, H=2048, I=8192, fp16.

Strategy: expert-parallel over 8 cores (2 experts/core), SPMD. Host
pre-transposes weights so the contraction dim is on SBUF partitions;
device streams weights (134 MB/core) at full DMA rate — the kernel is
HBM-bandwidth-bound (~320-375 us/core roofline at 360-420 GB/s).

Per expert on-device:
  fc1: lhsT = xT chunks [128, 64] (stationary), rhs = wiT tiles
       [128, 2048] (4 KB contiguous rows — measured 420 GB/s vs ~380
       for 2 KB rows), accumulate [64, 1024] in two PSUM banks over 16
       K-chunks; bias added via a one-hot-selector matmul.
  transpose: PE-transpose h [64, I] -> hT tiles [128, 64] (identity matmul).
  fc2: lhsT = hT chunks [128, 64], rhs = woT tiles [128, 2048],
       accumulate [64, 2048] in 4 PSUM banks over 64 K-chunks + bias.

Bias trick: SEL [128, G*64] holds one-hot column blocks (block g has row
g = 1, rest 0). matmul(ps, SEL[:, g*64:(g+1)*64], vals[:, :512]) adds
vals[g, :] (= bias[g*512:(g+1)*512]) to every output row. Per-expert
vals tiles kill cross-expert WAR chains on the in-order SP DMA queue.

Output DMA goes out on the ACT HWDGE ring (nc.scalar.dma_start) so its
dependency on the PSUM drain never stalls the SP weight-stream ring.
"""

import os
from contextlib import ExitStack

import numpy as np

E, B, C, H, I = 16, 1, 64, 2048, 8192
N_CORES = 8
E_LOC = E // N_CORES

_CACHE = {}


def build_program(e_loc=E_LOC, c=C, h=H, i=I, wi_bufs=25, wo_bufs=10,
                  fc1_interleave=False):
    import concourse.bass as bass
    import concourse.tile as tile
    from concourse import bacc, mybir
    from concourse.masks import make_identity

    fp16 = mybir.dt.float16
    fp32 = mybir.dt.float32

    assert c == 64 and h % 512 == 0 and i % 1024 == 0
    KH = h // 128          # fc1 contraction chunks
    KI = i // 128          # fc2 contraction chunks
    WI_TILE = 2048 if i % 2048 == 0 else 1024
    N_IGRP = i // WI_TILE
    PS1_W = 1024           # fc1 psum block width (2 banks)
    SUBS = WI_TILE // PS1_W
    NH = h // 512          # fc2 output column blocks
    TP_PER = 16 if KI % 16 == 0 else KI   # transposes per psum staging tile
    N_TGRP = KI // TP_PER
    G_WI = i // 512        # bias selector blocks for fc1
    G = max(G_WI, NH)

    nc = bacc.Bacc(
        "TRN2",
        target_bir_lowering=False,
        debug=False,
        enable_asserts=False,
        num_devices=N_CORES,
    )

    xt_ap = nc.dram_tensor("xt", [e_loc, 128, KH * c], fp16, kind="ExternalInput").ap()
    wiT_ap = nc.dram_tensor("wiT", [e_loc, h, i], fp16, kind="ExternalInput").ap()
    wib_ap = nc.dram_tensor("wib", [e_loc, G_WI, 512], fp16, kind="ExternalInput").ap()
    woT_ap = nc.dram_tensor("woT", [e_loc, i, h], fp16, kind="ExternalInput").ap()
    wob_ap = nc.dram_tensor("wob", [e_loc, NH, 512], fp16, kind="ExternalInput").ap()
    out_ap = nc.dram_tensor("out", [e_loc, c, h], fp16, kind="ExternalOutput").ap()

    with tile.TileContext(nc) as tc, ExitStack() as ctx:
        const_pool = ctx.enter_context(tc.tile_pool(name="const", bufs=1))
        xt_pool = ctx.enter_context(tc.tile_pool(name="xt", bufs=2))
        wi_pool = ctx.enter_context(tc.tile_pool(name="wi", bufs=wi_bufs))
        wo_pool = ctx.enter_context(tc.tile_pool(name="wo", bufs=wo_bufs))
        h_pool = ctx.enter_context(tc.tile_pool(name="h", bufs=1))
        ht_pool = ctx.enter_context(tc.tile_pool(name="ht", bufs=1))
        osb_pool = ctx.enter_context(tc.tile_pool(name="osb", bufs=2))
        ps1_pool = ctx.enter_context(tc.tile_pool(name="ps1", bufs=2, space="PSUM"))
        ps2_pool = ctx.enter_context(tc.tile_pool(name="ps2", bufs=1, space="PSUM"))

        ident = const_pool.tile([128, 128], fp16, tag="ident")
        make_identity(nc, ident)

        # one-hot selector: column block g has row g = 1, all else 0
        sel = const_pool.tile([128, G * c], fp16, tag="sel")
        nc.gpsimd.memset(sel, 0.0)
        sel3 = sel.rearrange("p (g c) -> p g c", c=c)
        nc.gpsimd.affine_select(
            out=sel3,
            in_=sel3,
            compare_op=mybir.AluOpType.not_equal,
            fill=1.0,
            base=0,
            # iota(p, g, c) = p - g; rows where p == g get fill=1.0
            pattern=[[-1, G], [0, c]],
            channel_multiplier=1,
        )

        # per-expert packed bias values (row g = bias[g*512:(g+1)*512])
        bwi = []
        bwo = []
        for e in range(e_loc):
            t = const_pool.tile([128, 512], fp16, tag=f"bwi{e}")
            nc.gpsimd.memset(t[:], 0.0)
            nc.gpsimd.dma_start(t[0:G_WI, :], wib_ap[e])
            bwi.append(t)
            t = const_pool.tile([128, 512], fp16, tag=f"bwo{e}")
            nc.gpsimd.memset(t[:], 0.0)
            nc.gpsimd.dma_start(t[0:NH, :], wob_ap[e])
            bwo.append(t)

        # both experts' activations up front (SWDGE; keeps the SP HWDGE
        # ring 100% weight traffic so its flow-control lanes never stall)
        xts = []
        for e in range(e_loc):
            xt_sb = xt_pool.tile([128, KH * c], fp16, tag="xt")
            nc.gpsimd.dma_start(xt_sb[:], xt_ap[e])
            xts.append(xt_sb)

        for e in range(e_loc):
            xt_sb = xts[e]
            h_sb = h_pool.tile([c, i], fp16, tag="h")

            # ---- fc1: h = x @ wiT + bi ----
            # Two live [64, PS1_W] accumulators per weight group so every
            # wi tile is fully consumed (all SUBS column blocks) the moment
            # it arrives — slot frees track DMA pace instead of bursting at
            # group end (which starved the DMA ~8 us/group).
            for ig in range(N_IGRP):
                if fc1_interleave:
                    pss = [ps1_pool.tile([c, PS1_W], fp32, tag="ps1",
                                         name=f"ps1_{e}_{ig}_{s}")
                           for s in range(SUBS)]
                    for k in range(KH):
                        wt = wi_pool.tile([128, WI_TILE], fp16, tag="wi")
                        nc.sync.dma_start(
                            wt[:],
                            wiT_ap[e, k * 128 : (k + 1) * 128,
                                   ig * WI_TILE : (ig + 1) * WI_TILE],
                        )
                        for sub in range(SUBS):
                            for q in range(PS1_W // 512):
                                nc.tensor.matmul(
                                    pss[sub][:, q * 512 : (q + 1) * 512],
                                    xt_sb[:, k * c : (k + 1) * c],
                                    wt[:, sub * PS1_W + q * 512
                                       : sub * PS1_W + (q + 1) * 512],
                                    start=(k == 0),
                                    stop=False,
                                )
                    for sub in range(SUBS):
                        off = ig * WI_TILE + sub * PS1_W
                        for q in range(PS1_W // 512):
                            g = off // 512 + q
                            nc.tensor.matmul(
                                pss[sub][:, q * 512 : (q + 1) * 512],
                                sel[:, g * c : (g + 1) * c],
                                bwi[e][:],
                                start=False,
                                stop=True,
                            )
                        nc.scalar.copy(h_sb[:, off : off + PS1_W], pss[sub][:])
                else:
                    witiles = []
                    for k in range(KH):
                        wt = wi_pool.tile([128, WI_TILE], fp16, tag="wi")
                        nc.sync.dma_start(
                            wt[:],
                            wiT_ap[e, k * 128 : (k + 1) * 128,
                                   ig * WI_TILE : (ig + 1) * WI_TILE],
                        )
                        witiles.append(wt)
                    for sub in range(SUBS):
                        off = ig * WI_TILE + sub * PS1_W
                        ps = ps1_pool.tile([c, PS1_W], fp32, tag="ps1")
                        for k in range(KH):
                            for q in range(PS1_W // 512):
                                nc.tensor.matmul(
                                    ps[:, q * 512 : (q + 1) * 512],
                                    xt_sb[:, k * c : (k + 1) * c],
                                    witiles[k][:, sub * PS1_W + q * 512
                                               : sub * PS1_W + (q + 1) * 512],
                                    start=(k == 0),
                                    stop=False,
                                )
                        for q in range(PS1_W // 512):
                            g = off // 512 + q
                            nc.tensor.matmul(
                                ps[:, q * 512 : (q + 1) * 512],
                                sel[:, g * c : (g + 1) * c],
                                bwi[e][:],
                                start=False,
                                stop=True,
                            )
                        nc.scalar.copy(h_sb[:, off : off + PS1_W], ps[:])

            # ---- transpose h -> hT ----
            ht_sb = ht_pool.tile([128, KI * c], fp16, tag="ht")
            for tg in range(N_TGRP):
                pst = ps1_pool.tile([128, TP_PER * c], fp16, tag="ps1")
                for j in range(TP_PER):
                    jj = tg * TP_PER + j
                    nc.tensor.transpose(
                        pst[:, j * c : (j + 1) * c],
                        h_sb[:, jj * 128 : (jj + 1) * 128],
                        ident[:c, :c],
                    )
                nc.vector.tensor_copy(
                    ht_sb[:, tg * TP_PER * c : (tg + 1) * TP_PER * c], pst[:]
                )

            # ---- fc2: out = h @ woT + bo ----
            pso = ps2_pool.tile([c, h], fp32, tag="ps2")
            for k in range(KI):
                wot = wo_pool.tile([128, h], fp16, tag="wo")
                nc.sync.dma_start(wot[:], woT_ap[e, k * 128 : (k + 1) * 128, :])
                for n in range(NH):
                    nc.tensor.matmul(
                        pso[:, n * 512 : (n + 1) * 512],
                        ht_sb[:, k * c : (k + 1) * c],
                        wot[:, n * 512 : (n + 1) * 512],
                        start=(k == 0),
                        stop=False,
                    )
            # Last expert: nothing left to stall, so use the idle ACT HWDGE
            # ring (faster issue than SWDGE) and alternate drain engines.
            # Earlier experts: SWDGE, so the late out completions never
            # block the SP weight-stream ring's flow-control lanes.
            last = e == e_loc - 1
            for n in range(NH):
                nc.tensor.matmul(
                    pso[:, n * 512 : (n + 1) * 512],
                    sel[:, n * c : (n + 1) * c],
                    bwo[e][:],
                    start=False,
                    stop=True,
                )
                out_sb = osb_pool.tile([c, 512], fp16, tag="osb")
                if last and n % 2 == 0:
                    nc.scalar.copy(out_sb[:], pso[:, n * 512 : (n + 1) * 512])
                else:
                    nc.vector.tensor_copy(out_sb[:], pso[:, n * 512 : (n + 1) * 512])
                eng = nc.scalar if last else nc.gpsimd
                eng.dma_start(out_ap[e, :, n * 512 : (n + 1) * 512], out_sb[:])

    nc.compile()
    return nc


def _get_program():
    key = (E_LOC, C, H, I)
    if key not in _CACHE:
        _CACHE[key] = build_program()
    return _CACHE[key]


def _make_in_maps(inputs, wi_w, wi_b, wo_w, wo_b):
    x = np.asarray(inputs, dtype=np.float16).reshape(E, C, H)
    # xt[e, p, k*C+c] = x[e, c, k*128+p]
    xt = np.ascontiguousarray(
        x.transpose(0, 2, 1).reshape(E, H // 128, 128, C)
        .transpose(0, 2, 1, 3).reshape(E, 128, (H // 128) * C)
    )
    wiT = np.ascontiguousarray(
        np.asarray(wi_w, dtype=np.float16).transpose(0, 2, 1)
    )  # [E, H, I]
    woT = np.ascontiguousarray(
        np.asarray(wo_w, dtype=np.float16).transpose(0, 2, 1)
    )  # [E, I, H]
    wib = np.ascontiguousarray(np.asarray(wi_b, dtype=np.float16)).reshape(E, I // 512, 512)
    wob = np.ascontiguousarray(np.asarray(wo_b, dtype=np.float16)).reshape(E, H // 512, 512)

    in_maps = []
    for r in range(N_CORES):
        s = slice(r * E_LOC, (r + 1) * E_LOC)
        in_maps.append(
            {
                "xt": np.ascontiguousarray(xt[s]),
                "wiT": np.ascontiguousarray(wiT[s]),
                "wib": np.ascontiguousarray(wib[s]),
                "woT": np.ascontiguousarray(woT[s]),
                "wob": np.ascontiguousarray(wob[s]),
            }
        )
    return in_maps


def run(inputs, wi_w, wi_b, wo_w, wo_b, trace=False):
    """Returns (output [E,B,C,H] fp16, exec_time_ns or None)."""
    from concourse.bass_utils import run_bass_kernel_spmd

    nc = _get_program()
    in_maps = _make_in_maps(inputs, wi_w, wi_b, wo_w, wo_b)
    res = run_bass_kernel_spmd(nc, in_maps, list(range(N_CORES)), trace=trace)
    out = np.stack([res.results[r]["out"] for r in range(N_CORES)])
    out = out.reshape(E, B, C, H).astype(np.float16)
    return out, res.exec_time_ns


def kernel(inputs, wi_w, wi_b, wo_w, wo_b):
    out, _ = run(inputs, wi_w, wi_b, wo_w, wo_b, trace=False)
    return out


# revision 14
# speedup vs baseline: 1.2346x; 1.0817x over previous
"""Trainium2 Bass kernel for BRT fused experts (grouped GEMM pair, no activation).

Reference semantics (per expert e):
    h   = x[e] @ wi_w[e].T + wi_b[e]        # [C, H] @ [H, I] -> [C, I]
    out = h @ wo_w[e].T + wo_b[e]           # [C, I] @ [I, H] -> [C, H]

Full dims: E=16, B=1, C=64, H=2048, I=8192, fp16.

Strategy: expert-parallel over 8 cores (2 experts/core), SPMD. Host
pre-transposes weights so the contraction dim is on SBUF partitions;
device streams weights (134 MB/core) at full DMA rate — the kernel is
HBM-bandwidth-bound (~320-375 us/core roofline at 360-420 GB/s).

Per expert on-device:
  fc1: lhsT = xT chunks [128, 64] (stationary), rhs = wiT tiles
       [128, 2048] (4 KB contiguous rows — measured 420 GB/s vs ~380
       for 2 KB rows), accumulate [64, 1024] in two PSUM banks over 16
       K-chunks; bias added via a one-hot-selector matmul.
  transpose: PE-transpose h [64, I] -> hT tiles [128, 64] (identity matmul).
  fc2: lhsT = hT chunks [128, 64], rhs = woT tiles [128, 2048],
       accumulate [64, 2048] in 4 PSUM banks over 64 K-chunks + bias.

Bias trick: SEL [128, G*64] holds one-hot column blocks (block g has row
g = 1, rest 0). matmul(ps, SEL[:, g*64:(g+1)*64], vals[:, :512]) adds
vals[g, :] (= bias[g*512:(g+1)*512]) to every output row. Per-expert
vals tiles kill cross-expert WAR chains on the in-order SP DMA queue.

Output DMA goes out on the ACT HWDGE ring (nc.scalar.dma_start) so its
dependency on the PSUM drain never stalls the SP weight-stream ring.
"""

import os
from contextlib import ExitStack

import numpy as np

E, B, C, H, I = 16, 1, 64, 2048, 8192
N_CORES = 8
E_LOC = E // N_CORES

_CACHE = {}


def build_program(e_loc=E_LOC, c=C, h=H, i=I, wi_bufs=25, wo_bufs=10,
                  fc1_interleave=False):
    import concourse.bass as bass
    import concourse.tile as tile
    from concourse import bacc, mybir
    from concourse.masks import make_identity

    fp16 = mybir.dt.float16
    fp32 = mybir.dt.float32

    assert c == 64 and h % 512 == 0 and i % 1024 == 0
    KH = h // 128          # fc1 contraction chunks
    KI = i // 128          # fc2 contraction chunks
    WI_TILE = 2048 if i % 2048 == 0 else 1024
    N_IGRP = i // WI_TILE
    PS1_W = 1024           # fc1 psum block width (2 banks)
    SUBS = WI_TILE // PS1_W
    NH = h // 512          # fc2 output column blocks
    TP_PER = 16 if KI % 16 == 0 else KI   # transposes per psum staging tile
    N_TGRP = KI // TP_PER
    G_WI = i // 512        # bias selector blocks for fc1
    G = max(G_WI, NH)

    nc = bacc.Bacc(
        "TRN2",
        target_bir_lowering=False,
        debug=False,
        enable_asserts=False,
        num_devices=N_CORES,
    )

    xt_ap = nc.dram_tensor("xt", [e_loc, 128, KH * c], fp16, kind="ExternalInput").ap()
    wiT_ap = nc.dram_tensor("wiT", [e_loc, h, i], fp16, kind="ExternalInput").ap()
    wib_ap = nc.dram_tensor("wib", [e_loc, G_WI, 512], fp16, kind="ExternalInput").ap()
    woT_ap = nc.dram_tensor("woT", [e_loc, i, h], fp16, kind="ExternalInput").ap()
    wob_ap = nc.dram_tensor("wob", [e_loc, NH, 512], fp16, kind="ExternalInput").ap()
    out_ap = nc.dram_tensor("out", [e_loc, c, h], fp16, kind="ExternalOutput").ap()

    with tile.TileContext(nc) as tc, ExitStack() as ctx:
        const_pool = ctx.enter_context(tc.tile_pool(name="const", bufs=1))
        xt_pool = ctx.enter_context(tc.tile_pool(name="xt", bufs=2))
        wi_pool = ctx.enter_context(tc.tile_pool(name="wi", bufs=wi_bufs))
        wo_pool = ctx.enter_context(tc.tile_pool(name="wo", bufs=wo_bufs))
        h_pool = ctx.enter_context(tc.tile_pool(name="h", bufs=1))
        ht_pool = ctx.enter_context(tc.tile_pool(name="ht", bufs=1))
        osb_pool = ctx.enter_context(tc.tile_pool(name="osb", bufs=6))
        ps1_pool = ctx.enter_context(tc.tile_pool(name="ps1", bufs=2, space="PSUM"))
        ps2_pool = ctx.enter_context(tc.tile_pool(name="ps2", bufs=1, space="PSUM"))

        ident = const_pool.tile([128, 128], fp16, tag="ident")
        make_identity(nc, ident)

        # one-hot selector: column block g has row g = 1, all else 0
        sel = const_pool.tile([128, G * c], fp16, tag="sel")
        nc.gpsimd.memset(sel, 0.0)
        sel3 = sel.rearrange("p (g c) -> p g c", c=c)
        nc.gpsimd.affine_select(
            out=sel3,
            in_=sel3,
            compare_op=mybir.AluOpType.not_equal,
            fill=1.0,
            base=0,
            # iota(p, g, c) = p - g; rows where p == g get fill=1.0
            pattern=[[-1, G], [0, c]],
            channel_multiplier=1,
        )

        # per-expert packed bias values (row g = bias[g*512:(g+1)*512])
        bwi = []
        bwo = []
        for e in range(e_loc):
            t = const_pool.tile([128, 512], fp16, tag=f"bwi{e}")
            nc.gpsimd.memset(t[:], 0.0)
            nc.gpsimd.dma_start(t[0:G_WI, :], wib_ap[e])
            bwi.append(t)
            t = const_pool.tile([128, 512], fp16, tag=f"bwo{e}")
            nc.gpsimd.memset(t[:], 0.0)
            nc.gpsimd.dma_start(t[0:NH, :], wob_ap[e])
            bwo.append(t)

        # both experts' activations up front (SWDGE; keeps the SP HWDGE
        # ring 100% weight traffic so its flow-control lanes never stall)
        xts = []
        for e in range(e_loc):
            xt_sb = xt_pool.tile([128, KH * c], fp16, tag="xt")
            nc.gpsimd.dma_start(xt_sb[:], xt_ap[e])
            xts.append(xt_sb)

        for e in range(e_loc):
            xt_sb = xts[e]
            h_sb = h_pool.tile([c, i], fp16, tag="h")

            # ---- fc1: h = x @ wiT + bi ----
            # Two live [64, PS1_W] accumulators per weight group so every
            # wi tile is fully consumed (all SUBS column blocks) the moment
            # it arrives — slot frees track DMA pace instead of bursting at
            # group end (which starved the DMA ~8 us/group).
            for ig in range(N_IGRP):
                if fc1_interleave:
                    pss = [ps1_pool.tile([c, PS1_W], fp32, tag="ps1",
                                         name=f"ps1_{e}_{ig}_{s}")
                           for s in range(SUBS)]
                    for k in range(KH):
                        wt = wi_pool.tile([128, WI_TILE], fp16, tag="wi")
                        nc.sync.dma_start(
                            wt[:],
                            wiT_ap[e, k * 128 : (k + 1) * 128,
                                   ig * WI_TILE : (ig + 1) * WI_TILE],
                        )
                        for sub in range(SUBS):
                            for q in range(PS1_W // 512):
                                nc.tensor.matmul(
                                    pss[sub][:, q * 512 : (q + 1) * 512],
                                    xt_sb[:, k * c : (k + 1) * c],
                                    wt[:, sub * PS1_W + q * 512
                                       : sub * PS1_W + (q + 1) * 512],
                                    start=(k == 0),
                                    stop=False,
                                )
                    for sub in range(SUBS):
                        off = ig * WI_TILE + sub * PS1_W
                        for q in range(PS1_W // 512):
                            g = off // 512 + q
                            nc.tensor.matmul(
                                pss[sub][:, q * 512 : (q + 1) * 512],
                                sel[:, g * c : (g + 1) * c],
                                bwi[e][:],
                                start=False,
                                stop=True,
                            )
                        nc.scalar.copy(h_sb[:, off : off + PS1_W], pss[sub][:])
                else:
                    witiles = []
                    for k in range(KH):
                        wt = wi_pool.tile([128, WI_TILE], fp16, tag="wi")
                        nc.sync.dma_start(
                            wt[:],
                            wiT_ap[e, k * 128 : (k + 1) * 128,
                                   ig * WI_TILE : (ig + 1) * WI_TILE],
                        )
                        witiles.append(wt)
                    for sub in range(SUBS):
                        off = ig * WI_TILE + sub * PS1_W
                        ps = ps1_pool.tile([c, PS1_W], fp32, tag="ps1")
                        for k in range(KH):
                            for q in range(PS1_W // 512):
                                nc.tensor.matmul(
                                    ps[:, q * 512 : (q + 1) * 512],
                                    xt_sb[:, k * c : (k + 1) * c],
                                    witiles[k][:, sub * PS1_W + q * 512
                                               : sub * PS1_W + (q + 1) * 512],
                                    start=(k == 0),
                                    stop=False,
                                )
                        for q in range(PS1_W // 512):
                            g = off // 512 + q
                            nc.tensor.matmul(
                                ps[:, q * 512 : (q + 1) * 512],
                                sel[:, g * c : (g + 1) * c],
                                bwi[e][:],
                                start=False,
                                stop=True,
                            )
                        nc.scalar.copy(h_sb[:, off : off + PS1_W], ps[:])

            # ---- transpose h -> hT ----
            ht_sb = ht_pool.tile([128, KI * c], fp16, tag="ht")
            for tg in range(N_TGRP):
                pst = ps1_pool.tile([128, TP_PER * c], fp16, tag="ps1")
                for j in range(TP_PER):
                    jj = tg * TP_PER + j
                    nc.tensor.transpose(
                        pst[:, j * c : (j + 1) * c],
                        h_sb[:, jj * 128 : (jj + 1) * 128],
                        ident[:c, :c],
                    )
                nc.vector.tensor_copy(
                    ht_sb[:, tg * TP_PER * c : (tg + 1) * TP_PER * c], pst[:]
                )

            # ---- fc2: out = h @ woT + bo ----
            pso = ps2_pool.tile([c, h], fp32, tag="ps2")
            for k in range(KI):
                wot = wo_pool.tile([128, h], fp16, tag="wo")
                nc.sync.dma_start(wot[:], woT_ap[e, k * 128 : (k + 1) * 128, :])
                for n in range(NH):
                    nc.tensor.matmul(
                        pso[:, n * 512 : (n + 1) * 512],
                        ht_sb[:, k * c : (k + 1) * c],
                        wot[:, n * 512 : (n + 1) * 512],
                        start=(k == 0),
                        stop=False,
                    )
            # Last expert: nothing left to stall, so use the idle ACT HWDGE
            # ring (faster issue than SWDGE) and alternate drain engines.
            # Earlier experts: SWDGE, so the late out completions never
            # block the SP weight-stream ring's flow-control lanes.
            last = e == e_loc - 1
            for n in range(NH):
                nc.tensor.matmul(
                    pso[:, n * 512 : (n + 1) * 512],
                    sel[:, n * c : (n + 1) * c],
                    bwo[e][:],
                    start=False,
                    stop=True,
                )
                out_sb = osb_pool.tile([c, 512], fp16, tag="osb")
                if last and n % 2 == 0:
                    nc.scalar.copy(out_sb[:], pso[:, n * 512 : (n + 1) * 512])
                else:
                    nc.vector.tensor_copy(out_sb[:], pso[:, n * 512 : (n + 1) * 512])
                eng = nc.scalar if last else nc.gpsimd
                eng.dma_start(out_ap[e, :, n * 512 : (n + 1) * 512], out_sb[:])

    nc.compile()
    return nc


def _get_program():
    key = (E_LOC, C, H, I)
    if key not in _CACHE:
        _CACHE[key] = build_program()
    return _CACHE[key]


def _make_in_maps(inputs, wi_w, wi_b, wo_w, wo_b):
    x = np.asarray(inputs, dtype=np.float16).reshape(E, C, H)
    # xt[e, p, k*C+c] = x[e, c, k*128+p]
    xt = np.ascontiguousarray(
        x.transpose(0, 2, 1).reshape(E, H // 128, 128, C)
        .transpose(0, 2, 1, 3).reshape(E, 128, (H // 128) * C)
    )
    wiT = np.ascontiguousarray(
        np.asarray(wi_w, dtype=np.float16).transpose(0, 2, 1)
    )  # [E, H, I]
    woT = np.ascontiguousarray(
        np.asarray(wo_w, dtype=np.float16).transpose(0, 2, 1)
    )  # [E, I, H]
    wib = np.ascontiguousarray(np.asarray(wi_b, dtype=np.float16)).reshape(E, I // 512, 512)
    wob = np.ascontiguousarray(np.asarray(wo_b, dtype=np.float16)).reshape(E, H // 512, 512)

    in_maps = []
    for r in range(N_CORES):
        s = slice(r * E_LOC, (r + 1) * E_LOC)
        in_maps.append(
            {
                "xt": np.ascontiguousarray(xt[s]),
                "wiT": np.ascontiguousarray(wiT[s]),
                "wib": np.ascontiguousarray(wib[s]),
                "woT": np.ascontiguousarray(woT[s]),
                "wob": np.ascontiguousarray(wob[s]),
            }
        )
    return in_maps


def run(inputs, wi_w, wi_b, wo_w, wo_b, trace=False):
    """Returns (output [E,B,C,H] fp16, exec_time_ns or None)."""
    from concourse.bass_utils import run_bass_kernel_spmd

    nc = _get_program()
    in_maps = _make_in_maps(inputs, wi_w, wi_b, wo_w, wo_b)
    res = run_bass_kernel_spmd(nc, in_maps, list(range(N_CORES)), trace=trace)
    out = np.stack([res.results[r]["out"] for r in range(N_CORES)])
    out = out.reshape(E, B, C, H).astype(np.float16)
    return out, res.exec_time_ns


def kernel(inputs, wi_w, wi_b, wo_w, wo_b):
    out, _ = run(inputs, wi_w, wi_b, wo_w, wo_b, trace=False)
    return out


# revision 16
# speedup vs baseline: 1.2356x; 1.0008x over previous
"""Trainium2 Bass kernel for BRT fused experts (grouped GEMM pair, no activation).

Reference semantics (per expert e):
    h   = x[e] @ wi_w[e].T + wi_b[e]        # [C, H] @ [H, I] -> [C, I]
    out = h @ wo_w[e].T + wo_b[e]           # [C, I] @ [I, H] -> [C, H]

Full dims: E=16, B=1, C=64, H=2048, I=8192, fp16.

Strategy: expert-parallel over 8 cores (2 experts/core), SPMD. Host
pre-transposes weights so the contraction dim is on SBUF partitions;
device streams weights (134 MB/core) at full DMA rate — the kernel is
HBM-bandwidth-bound (~320-375 us/core roofline at 360-420 GB/s).

Per expert on-device:
  fc1: lhsT = xT chunks [128, 64] (stationary), rhs = wiT tiles
       [128, 2048] (4 KB contiguous rows — measured 420 GB/s vs ~380
       for 2 KB rows), accumulate [64, 1024] in two PSUM banks over 16
       K-chunks; bias added via a one-hot-selector matmul.
  transpose: PE-transpose h [64, I] -> hT tiles [128, 64] (identity matmul).
  fc2: lhsT = hT chunks [128, 64], rhs = woT tiles [128, 2048],
       accumulate [64, 2048] in 4 PSUM banks over 64 K-chunks + bias.

Bias trick: SEL [128, G*64] holds one-hot column blocks (block g has row
g = 1, rest 0). matmul(ps, SEL[:, g*64:(g+1)*64], vals[:, :512]) adds
vals[g, :] (= bias[g*512:(g+1)*512]) to every output row. Per-expert
vals tiles kill cross-expert WAR chains on the in-order SP DMA queue.

Non-weight DMAs (activations, biases, earlier experts' outputs) ride
SWDGE (gpsimd) so their slow completions never occupy the HWDGE
flow-control lanes that pace the SP weight stream; the last expert's
output uses the idle ACT HWDGE ring for a shorter kernel tail.

Measured on 8 axon TRN2 cores: ~364 us (good rounds), vs ~330 us pure
DMA floor at the sustained 410 GB/s/core rate — DMA busy 94-96%.
"""

from contextlib import ExitStack

import numpy as np

E, B, C, H, I = 16, 1, 64, 2048, 8192
N_CORES = 8
E_LOC = E // N_CORES

_CACHE = {}


def build_program(e_loc=E_LOC, c=C, h=H, i=I, wi_bufs=25, wo_bufs=10,
                  fc1_interleave=False):
    import concourse.bass as bass
    import concourse.tile as tile
    from concourse import bacc, mybir
    from concourse.masks import make_identity

    fp16 = mybir.dt.float16
    fp32 = mybir.dt.float32

    assert c == 64 and h % 512 == 0 and i % 1024 == 0
    KH = h // 128          # fc1 contraction chunks
    KI = i // 128          # fc2 contraction chunks
    WI_TILE = 2048 if i % 2048 == 0 else 1024
    N_IGRP = i // WI_TILE
    PS1_W = 1024           # fc1 psum block width (2 banks)
    SUBS = WI_TILE // PS1_W
    NH = h // 512          # fc2 output column blocks
    TP_PER = 16 if KI % 16 == 0 else KI   # transposes per psum staging tile
    N_TGRP = KI // TP_PER
    G_WI = i // 512        # bias selector blocks for fc1
    G = max(G_WI, NH)

    nc = bacc.Bacc(
        "TRN2",
        target_bir_lowering=False,
        debug=False,
        enable_asserts=False,
        num_devices=N_CORES,
    )

    xt_ap = nc.dram_tensor("xt", [e_loc, 128, KH * c], fp16, kind="ExternalInput").ap()
    wiT_ap = nc.dram_tensor("wiT", [e_loc, h, i], fp16, kind="ExternalInput").ap()
    wib_ap = nc.dram_tensor("wib", [e_loc, G_WI, 512], fp16, kind="ExternalInput").ap()
    woT_ap = nc.dram_tensor("woT", [e_loc, i, h], fp16, kind="ExternalInput").ap()
    wob_ap = nc.dram_tensor("wob", [e_loc, NH, 512], fp16, kind="ExternalInput").ap()
    out_ap = nc.dram_tensor("out", [e_loc, c, h], fp16, kind="ExternalOutput").ap()

    with tile.TileContext(nc) as tc, ExitStack() as ctx:
        const_pool = ctx.enter_context(tc.tile_pool(name="const", bufs=1))
        xt_pool = ctx.enter_context(tc.tile_pool(name="xt", bufs=2))
        wi_pool = ctx.enter_context(tc.tile_pool(name="wi", bufs=wi_bufs))
        wo_pool = ctx.enter_context(tc.tile_pool(name="wo", bufs=wo_bufs))
        h_pool = ctx.enter_context(tc.tile_pool(name="h", bufs=1))
        ht_pool = ctx.enter_context(tc.tile_pool(name="ht", bufs=1))
        osb_pool = ctx.enter_context(tc.tile_pool(name="osb", bufs=6))
        ps1_pool = ctx.enter_context(tc.tile_pool(name="ps1", bufs=2, space="PSUM"))
        ps2_pool = ctx.enter_context(tc.tile_pool(name="ps2", bufs=1, space="PSUM"))

        ident = const_pool.tile([128, 128], fp16, tag="ident")
        make_identity(nc, ident)

        # one-hot selector: column block g has row g = 1, all else 0
        sel = const_pool.tile([128, G * c], fp16, tag="sel")
        nc.gpsimd.memset(sel, 0.0)
        sel3 = sel.rearrange("p (g c) -> p g c", c=c)
        nc.gpsimd.affine_select(
            out=sel3,
            in_=sel3,
            compare_op=mybir.AluOpType.not_equal,
            fill=1.0,
            base=0,
            # iota(p, g, c) = p - g; rows where p == g get fill=1.0
            pattern=[[-1, G], [0, c]],
            channel_multiplier=1,
        )

        # per-expert packed bias values (row g = bias[g*512:(g+1)*512])
        bwi = []
        bwo = []
        for e in range(e_loc):
            t = const_pool.tile([128, 512], fp16, tag=f"bwi{e}")
            nc.gpsimd.memset(t[:], 0.0)
            nc.gpsimd.dma_start(t[0:G_WI, :], wib_ap[e])
            bwi.append(t)
            t = const_pool.tile([128, 512], fp16, tag=f"bwo{e}")
            nc.gpsimd.memset(t[:], 0.0)
            nc.gpsimd.dma_start(t[0:NH, :], wob_ap[e])
            bwo.append(t)

        # both experts' activations up front (SWDGE; keeps the SP HWDGE
        # ring 100% weight traffic so its flow-control lanes never stall)
        xts = []
        for e in range(e_loc):
            xt_sb = xt_pool.tile([128, KH * c], fp16, tag="xt")
            nc.gpsimd.dma_start(xt_sb[:], xt_ap[e])
            xts.append(xt_sb)

        for e in range(e_loc):
            xt_sb = xts[e]
            h_sb = h_pool.tile([c, i], fp16, tag="h")

            # ---- fc1: h = x @ wiT + bi ----
            # Default (fc1_interleave=False): per weight group, DMA all 16
            # K-chunk tiles, then accumulate the SUBS column blocks in two
            # passes over them. (The interleaved single-pass variant
            # measured ~50 us slower in paired A/B — keep it False.)
            for ig in range(N_IGRP):
                if fc1_interleave:
                    pss = [ps1_pool.tile([c, PS1_W], fp32, tag="ps1",
                                         name=f"ps1_{e}_{ig}_{s}")
                           for s in range(SUBS)]
                    for k in range(KH):
                        wt = wi_pool.tile([128, WI_TILE], fp16, tag="wi")
                        nc.sync.dma_start(
                            wt[:],
                            wiT_ap[e, k * 128 : (k + 1) * 128,
                                   ig * WI_TILE : (ig + 1) * WI_TILE],
                        )
                        for sub in range(SUBS):
                            for q in range(PS1_W // 512):
                                nc.tensor.matmul(
                                    pss[sub][:, q * 512 : (q + 1) * 512],
                                    xt_sb[:, k * c : (k + 1) * c],
                                    wt[:, sub * PS1_W + q * 512
                                       : sub * PS1_W + (q + 1) * 512],
                                    start=(k == 0),
                                    stop=False,
                                )
                    for sub in range(SUBS):
                        off = ig * WI_TILE + sub * PS1_W
                        for q in range(PS1_W // 512):
                            g = off // 512 + q
                            nc.tensor.matmul(
                                pss[sub][:, q * 512 : (q + 1) * 512],
                                sel[:, g * c : (g + 1) * c],
                                bwi[e][:],
                                start=False,
                                stop=True,
                            )
                        nc.scalar.copy(h_sb[:, off : off + PS1_W], pss[sub][:])
                else:
                    witiles = []
                    for k in range(KH):
                        wt = wi_pool.tile([128, WI_TILE], fp16, tag="wi")
                        nc.sync.dma_start(
                            wt[:],
                            wiT_ap[e, k * 128 : (k + 1) * 128,
                                   ig * WI_TILE : (ig + 1) * WI_TILE],
                        )
                        witiles.append(wt)
                    for sub in range(SUBS):
                        off = ig * WI_TILE + sub * PS1_W
                        ps = ps1_pool.tile([c, PS1_W], fp32, tag="ps1")
                        for k in range(KH):
                            for q in range(PS1_W // 512):
                                nc.tensor.matmul(
                                    ps[:, q * 512 : (q + 1) * 512],
                                    xt_sb[:, k * c : (k + 1) * c],
                                    witiles[k][:, sub * PS1_W + q * 512
                                               : sub * PS1_W + (q + 1) * 512],
                                    start=(k == 0),
                                    stop=False,
                                )
                        for q in range(PS1_W // 512):
                            g = off // 512 + q
                            nc.tensor.matmul(
                                ps[:, q * 512 : (q + 1) * 512],
                                sel[:, g * c : (g + 1) * c],
                                bwi[e][:],
                                start=False,
                                stop=True,
                            )
                        nc.scalar.copy(h_sb[:, off : off + PS1_W], ps[:])

            # ---- transpose h -> hT ----
            ht_sb = ht_pool.tile([128, KI * c], fp16, tag="ht")
            for tg in range(N_TGRP):
                pst = ps1_pool.tile([128, TP_PER * c], fp16, tag="ps1")
                for j in range(TP_PER):
                    jj = tg * TP_PER + j
                    nc.tensor.transpose(
                        pst[:, j * c : (j + 1) * c],
                        h_sb[:, jj * 128 : (jj + 1) * 128],
                        ident[:c, :c],
                    )
                nc.vector.tensor_copy(
                    ht_sb[:, tg * TP_PER * c : (tg + 1) * TP_PER * c], pst[:]
                )

            # ---- fc2: out = h @ woT + bo ----
            pso = ps2_pool.tile([c, h], fp32, tag="ps2")
            for k in range(KI):
                wot = wo_pool.tile([128, h], fp16, tag="wo")
                nc.sync.dma_start(wot[:], woT_ap[e, k * 128 : (k + 1) * 128, :])
                for n in range(NH):
                    nc.tensor.matmul(
                        pso[:, n * 512 : (n + 1) * 512],
                        ht_sb[:, k * c : (k + 1) * c],
                        wot[:, n * 512 : (n + 1) * 512],
                        start=(k == 0),
                        stop=False,
                    )
            # Last expert: nothing left to stall, so use the idle ACT HWDGE
            # ring (faster issue than SWDGE) and alternate drain engines.
            # Earlier experts: SWDGE, so the late out completions never
            # block the SP weight-stream ring's flow-control lanes.
            last = e == e_loc - 1
            for n in range(NH):
                nc.tensor.matmul(
                    pso[:, n * 512 : (n + 1) * 512],
                    sel[:, n * c : (n + 1) * c],
                    bwo[e][:],
                    start=False,
                    stop=True,
                )
                out_sb = osb_pool.tile([c, 512], fp16, tag="osb")
                if last and n % 2 == 0:
                    nc.scalar.copy(out_sb[:], pso[:, n * 512 : (n + 1) * 512])
                else:
                    nc.vector.tensor_copy(out_sb[:], pso[:, n * 512 : (n + 1) * 512])
                eng = nc.scalar if last else nc.gpsimd
                eng.dma_start(out_ap[e, :, n * 512 : (n + 1) * 512], out_sb[:])

    nc.compile()
    return nc


def _get_program():
    key = (E_LOC, C, H, I)
    if key not in _CACHE:
        _CACHE[key] = build_program()
    return _CACHE[key]


def _make_in_maps(inputs, wi_w, wi_b, wo_w, wo_b):
    x = np.asarray(inputs, dtype=np.float16).reshape(E, C, H)
    # xt[e, p, k*C+c] = x[e, c, k*128+p]
    xt = np.ascontiguousarray(
        x.transpose(0, 2, 1).reshape(E, H // 128, 128, C)
        .transpose(0, 2, 1, 3).reshape(E, 128, (H // 128) * C)
    )
    wiT = np.ascontiguousarray(
        np.asarray(wi_w, dtype=np.float16).transpose(0, 2, 1)
    )  # [E, H, I]
    woT = np.ascontiguousarray(
        np.asarray(wo_w, dtype=np.float16).transpose(0, 2, 1)
    )  # [E, I, H]
    wib = np.ascontiguousarray(np.asarray(wi_b, dtype=np.float16)).reshape(E, I // 512, 512)
    wob = np.ascontiguousarray(np.asarray(wo_b, dtype=np.float16)).reshape(E, H // 512, 512)

    in_maps = []
    for r in range(N_CORES):
        s = slice(r * E_LOC, (r + 1) * E_LOC)
        in_maps.append(
            {
                "xt": np.ascontiguousarray(xt[s]),
                "wiT": np.ascontiguousarray(wiT[s]),
                "wib": np.ascontiguousarray(wib[s]),
                "woT": np.ascontiguousarray(woT[s]),
                "wob": np.ascontiguousarray(wob[s]),
            }
        )
    return in_maps


def run(inputs, wi_w, wi_b, wo_w, wo_b, trace=False):
    """Returns (output [E,B,C,H] fp16, exec_time_ns or None)."""
    from concourse.bass_utils import run_bass_kernel_spmd

    nc = _get_program()
    in_maps = _make_in_maps(inputs, wi_w, wi_b, wo_w, wo_b)
    res = run_bass_kernel_spmd(nc, in_maps, list(range(N_CORES)), trace=trace)
    out = np.stack([res.results[r]["out"] for r in range(N_CORES)])
    out = out.reshape(E, B, C, H).astype(np.float16)
    return out, res.exec_time_ns


def kernel(inputs, wi_w, wi_b, wo_w, wo_b):
    out, _ = run(inputs, wi_w, wi_b, wo_w, wo_b, trace=False)
    return out
